# revision 24
# baseline (speedup 1.0000x reference)
"""Trainium2 Bass kernel for nn_BertAttentionEx (BERT attention with
relative_key_query position embeddings + output dense + residual + LayerNorm).

Distribution: 8 cores = 4 batches x 2 head-groups (8 heads each).

v4 design:
  - The attention u-loop is split by sequence half (lh), which shrinks the
    PV accumulators to 2 PSUM banks and frees 2 banks as a dedicated band
    pipeline, so band computation for pair p+2 overlaps attention for pair
    p with zero PSUM coupling; the PE stream stays dense (HAM stays 8/8).
  - Score tiles are [128, 1024] with both heads side by side: one exp
    instruction per (u, lh) instead of four, one PSUM tile instead of two.
  - PV matmuls lag the score matmuls by one u-step so exp latency hides
    behind PE work.
  - Tensor-parallel exchange is a per-pair AllGather of ctx halves
    (overlapped into the next pair); the output dense runs at full K=1024
    on this core's sequence half (rank-dynamic read-back), then residual +
    LayerNorm locally.  No serial ReduceScatter tail.
"""
import sys
import numpy as np
import ml_dtypes
from contextlib import ExitStack

sys.path.insert(0, "/opt/trn_rl_repo")

import concourse.bass as bass
import concourse.bacc as bacc
import concourse.tile as tile
from concourse import mybir
from concourse.bass_utils import run_bass_kernel_spmd

B, S, HID = 4, 1024, 1024
NH, HD = 16, 64
MAX_POS = 1024
LN_EPS = 1e-12
NCORES = 8
HPC = 8           # heads per core
W = 1152          # band width per 128-row tile
BT = S // 128     # 8 row tiles
F32 = mybir.dt.float32
BF16 = mybir.dt.bfloat16
FP8 = mybir.dt.float8e4
AF = mybir.ActivationFunctionType
ALU = mybir.AluOpType

_COMPILED = None


def build_program():
    nc = bacc.Bacc("TRN2", target_bir_lowering=False, debug=False,
                   num_devices=NCORES)

    # ---- per-core external I/O (host pre-casts to bf16 where noted) ----
    hsT = nc.declare_dram_parameter("hsT", [HID, S], BF16, isOutput=False)
    res = nc.declare_dram_parameter("res", [S // 2, HID], BF16, isOutput=False)
    wqT = nc.declare_dram_parameter("wqT", [HID, 512], BF16, isOutput=False)
    wkT = nc.declare_dram_parameter("wkT", [HID, 512], BF16, isOutput=False)
    wvT = nc.declare_dram_parameter("wvT", [HID, 520], BF16, isOutput=False)
    bqv = nc.declare_dram_parameter("bq", [128, 4], F32, isOutput=False)
    bkv = nc.declare_dram_parameter("bk", [128, 4], F32, isOutput=False)
    bvaug = nc.declare_dram_parameter("bvaug", [520], F32, isOutput=False)
    drTt = nc.declare_dram_parameter("drT", [128, 2048], BF16, isOutput=False)
    dTt = nc.declare_dram_parameter("dT", [128, 2048], BF16, isOutput=False)
    woT = nc.declare_dram_parameter("woT", [HID, HID], BF16, isOutput=False)
    maskc = nc.declare_dram_parameter("maskc", [128, 8], F32, isOutput=False)
    ident8 = nc.declare_dram_parameter("ident8", [128, 128], FP8, isOutput=False)
    ones64 = nc.declare_dram_parameter("ones64", [128, 64], BF16, isOutput=False)
    lng = nc.declare_dram_parameter("lng", [HID], BF16, isOutput=False)
    lnb = nc.declare_dram_parameter("lnb", [HID], BF16, isOutput=False)
    out = nc.declare_dram_parameter("out", [S // 2, HID], F32, isOutput=True)

    # internal DRAM: fp8 bands, partition-major [p, t, j] (pitch 8*W per p)
    bandA = [nc.dram_tensor(f"bandA{i}", [128, BT, W], FP8) for i in range(8)]
    bandB = [nc.dram_tensor(f"bandB{i}", [128, BT, W], FP8) for i in range(8)]
    # ctx exchange: block h = both heads' ctx for seq-half h; AllGather
    # concatenates rank blocks; read back own half with rank-dynamic offset.
    ctx_send = [[nc.dram_tensor(f"ctx_send{p}_{h}", [128, 512], BF16)
                 for h in range(2)] for p in range(4)]
    zdram = [nc.dram_tensor(f"zdram{h}", [1, S], BF16) for h in range(2)]
    ctx_recv = [nc.dram_tensor(f"ctx_recv{p}", [512, 512], BF16)
                for p in range(4)]

    PIT = BT * W  # per-partition band pitch (elements)
    RG = [[0, 1], [2, 3], [4, 5], [6, 7]]

    with ExitStack() as ctx:
        tc = ctx.enter_context(tile.TileContext(nc))
        consts = ctx.enter_context(tc.tile_pool(name="consts", bufs=1))
        persist = ctx.enter_context(tc.tile_pool(name="persist", bufs=1))
        wpool = ctx.enter_context(tc.tile_pool(name="wpool", bufs=2))
        hsp = ctx.enter_context(tc.tile_pool(name="hsp", bufs=8))
        bandsb = ctx.enter_context(tc.tile_pool(name="bandsb", bufs=3))
        aexp = ctx.enter_context(tc.tile_pool(name="aexp", bufs=4))
        gkp = ctx.enter_context(tc.tile_pool(name="gkp", bufs=6))
        ppool = ctx.enter_context(tc.tile_pool(name="ppool", bufs=2))
        ctxph = [ctx.enter_context(tc.tile_pool(name=f"ctxph{i}", bufs=2))
                 for i in range(2)]
        misc = ctx.enter_context(tc.tile_pool(name="misc", bufs=2))
        ctxup = ctx.enter_context(tc.tile_pool(name="ctxup", bufs=2))
        bigp = ctx.enter_context(tc.tile_pool(name="bigp", bufs=2))
        lns = ctx.enter_context(tc.tile_pool(name="lns", bufs=2))
        # PSUM: 4 banks scores/proj/Wo ([128,1024] x2), 2 banks bands
        # ([128,512] x2), 2 banks PV accumulators ([65,512] x2)
        psS = ctx.enter_context(tc.tile_pool(name="psS", bufs=2, space="PSUM"))
        psB = ctx.enter_context(tc.tile_pool(name="psB", bufs=2, space="PSUM"))
        psC = ctx.enter_context(tc.tile_pool(name="psC", bufs=1, space="PSUM"))

        # ---- small constants ----
        bq_sb = consts.tile([128, 4], F32)
        nc.sync.dma_start(out=bq_sb, in_=bqv[:, :])
        bk_sb = consts.tile([128, 4], F32)
        nc.sync.dma_start(out=bk_sb, in_=bkv[:, :])
        bv_bc = consts.tile([128, 520], F32)
        nc.sync.dma_start(
            out=bv_bc,
            in_=bass.AP(tensor=bvaug, offset=0, ap=[[0, 128], [1, 520]]),
        )
        mask_sb = consts.tile([128, 8], F32)
        nc.sync.dma_start(out=mask_sb, in_=maskc[:, :])
        id8_sb = consts.tile([128, 128], FP8)
        nc.sync.dma_start(out=id8_sb, in_=ident8[:, :])
        ones_row = consts.tile([128, 64], BF16)
        nc.sync.dma_start(out=ones_row, in_=ones64[:, :])
        lng_bc = consts.tile([128, HID], BF16)
        nc.gpsimd.dma_start(
            out=lng_bc,
            in_=bass.AP(tensor=lng, offset=0, ap=[[0, 128], [1, HID]]),
        )
        lnb_bc = consts.tile([128, HID], BF16)
        nc.gpsimd.dma_start(
            out=lnb_bc,
            in_=bass.AP(tensor=lnb, offset=0, ap=[[0, 128], [1, HID]]),
        )
        eps_sb = consts.tile([128, 1], F32)
        nc.vector.memset(eps_sb, LN_EPS)

        # ---- persistent activations ----
        qT_sb = persist.tile([128, 4, S], BF16, tag="qT")    # [d, l], 2 heads/tile
        kT_sb = persist.tile([128, 4, S], BF16, tag="kT")
        vv_sb = persist.tile([128, 8, 520], BF16, tag="vv")  # v natural [r, 65h+..]
        ctx2 = persist.tile([128, 8, 512], BF16, tag="ctx2")  # Wo K-chunks

        # ---- prologue loads: interleave hs^T + wq + wk ----
        hsT_tiles = []
        wq_sb = wpool.tile([128, 8, 520], BF16, tag="w", name="wq_sb")
        wk_sb = wpool.tile([128, 8, 520], BF16, tag="w", name="wk_sb")
        for kc in range(8):
            htile = hsp.tile([128, S], BF16, tag="hst", name=f"hsT{kc}")
            nc.sync.dma_start(out=htile, in_=hsT[128 * kc:128 * kc + 128, :])
            hsT_tiles.append(htile)
            nc.sync.dma_start(out=wq_sb[:, kc, 0:512],
                              in_=wqT[128 * kc:128 * kc + 128, :])
            nc.sync.dma_start(out=wk_sb[:, kc, 0:512],
                              in_=wkT[128 * kc:128 * kc + 128, :])
        drT_sb = consts.tile([128, 2048], BF16)
        nc.sync.dma_start(out=drT_sb, in_=drTt[:, :])
        dT_sb = consts.tile([128, 2048], BF16)
        nc.sync.dma_start(out=dT_sb, in_=dTt[:, :])

        def proj_block(w_sb, b_sb, dst, i):
            # one pair-block of the q/k projection on the (prologue-idle)
            # psC banks, one group per column half
            for nh2 in range(2):
                ps = psC.tile([128, 512], F32, tag=f"c{nh2}",
                              name=f"ps_{dst.name}_{i}_{nh2}")
                for kc in range(8):
                    nc.tensor.matmul(
                        ps,
                        lhsT=w_sb[:, kc, 128 * i:128 * i + 128],
                        rhs=hsT_tiles[kc][:, 512 * nh2:512 * nh2 + 512],
                        start=(kc == 0), stop=(kc == 7),
                    )
                nc.scalar.activation(
                    out=dst[:, i, 512 * nh2:512 * nh2 + 512],
                    in_=ps, func=AF.Identity,
                    bias=b_sb[:, i:i + 1], scale=1.0,
                )

        def emit_v_block(rt):
            for ci, (c0, cn) in enumerate(((0, 260), (260, 260))):
                ps = psC.tile([128, 512], F32, tag=f"c{ci}",
                              name=f"ps_v_{rt}_{ci}")
                for kc in range(8):
                    nc.tensor.matmul(
                        ps[:, 0:cn],
                        lhsT=hsT_tiles[kc][:, 128 * rt:128 * rt + 128],
                        rhs=wv_sb[:, kc, c0:c0 + cn],
                        start=(kc == 0), stop=(kc == 7),
                    )
                nc.vector.tensor_tensor(
                    out=vv_sb[:, rt, c0:c0 + cn],
                    in0=ps[:, 0:cn], in1=bv_bc[:, c0:c0 + cn], op=ALU.add,
                )

        # ---- band machinery (dedicated psB banks) ----
        evac_rr = [0, 1, 0, 1, 0, 1, 0, 1, 0]  # 0 ACT (5/9), 1 DVE (4/9)
        evac_ctr = [0]
        band_stage = {}

        def _evac(dst, src_ps):
            eng = evac_rr[evac_ctr[0] % len(evac_rr)]
            evac_ctr[0] += 1
            if eng == 0:
                nc.scalar.copy(out=dst, in_=src_ps)
            else:
                nc.vector.tensor_copy(out=dst, in_=src_ps)

        def emit_band_t(hp, side, t, big=False):
            # one row-tile of the band for both heads of pair hp on one side.
            # big=True (prologue): [128,1024] main tiles on the then-idle psS
            # slots -> half the evac instructions.
            bsrc, table, bufs_ = ((qT_sb, drT_sb, bandA) if side == 0
                                  else (kT_sb, dT_sb, bandB))
            j0 = 896 - 128 * t
            th, tl = t // 2, t % 2
            if tl == 0:
                band_stage[side] = [
                    bandsb.tile([128, 2, W], FP8, tag="bandsb",
                                name=f"bst{hp}_{side}_{th}_{h}")
                    for h in range(2)]
            stg = band_stage[side]
            for hh in range(2):
                hb = 64 * hh
                lhsT = bsrc[hb:hb + 64, hp, 128 * t:128 * t + 128]
                if big:
                    psm = psS.tile([128, 1024], F32, tag="ps",
                                   name=f"psm{side}{hp}{hh}_{t}")
                    for c0 in (0, 512):
                        nc.tensor.matmul(
                            psm[:, c0:c0 + 512],
                            lhsT=lhsT,
                            rhs=table[hb:hb + 64, j0 + c0:j0 + c0 + 512],
                            start=True, stop=True,
                        )
                    _evac(stg[hh][:, tl, 0:1024], psm)
                    pst = psB.tile([128, 512], F32, tag="bp",
                                   name=f"pst{side}{hp}{hh}_{t}")
                    nc.tensor.matmul(
                        pst[:, 0:128],
                        lhsT=lhsT,
                        rhs=table[hb:hb + 64, j0 + 1024:j0 + 1152],
                        start=True, stop=True,
                    )
                    _evac(stg[hh][:, tl, 1024:1152], pst[:, 0:128])
                else:
                    for (c0, cn) in ((0, 512), (512, 512), (1024, 128)):
                        ps = psB.tile([128, 512], F32, tag="bp",
                                      name=f"psb{side}{hp}{hh}_{t}_{c0}")
                        nc.tensor.matmul(
                            ps[:, 0:cn],
                            lhsT=lhsT,
                            rhs=table[hb:hb + 64, j0 + c0:j0 + c0 + cn],
                            start=True, stop=True,
                        )
                        _evac(stg[hh][:, tl, c0:c0 + cn], ps[:, 0:cn])
            if tl == 1:
                for hh in range(2):
                    nc.sync.dma_start(
                        out=bass.AP(tensor=bufs_[2 * hp + hh],
                                    offset=th * 2 * W,
                                    ap=[[PIT, 128], [1, 2 * W]]),
                        in_=stg[hh],
                    )

        def issue_aex(hp):
            # q-side skewed gather (natural layout [l', t, r]), per head
            tiles = []
            for hh in range(2):
                aex = aexp.tile([128, BT, S], FP8, tag="aex",
                                name=f"aex{hp}_{hh}")
                nc.sync.dma_start(
                    out=aex,
                    in_=bass.AP(tensor=bandA[2 * hp + hh], offset=127,
                                ap=[[PIT - 1, 128], [W, BT], [1, S]]),
                )
                tiles.append(aex)
            return tiles

        def issue_gk(hp, uh):
            # k-side skewed gather for u-half uh (score^T layout [r', du, l])
            tiles = []
            for hh in range(2):
                gk = gkp.tile([128, 4, S], FP8, tag="gk",
                              name=f"gk{hp}_{uh}_{hh}")
                nc.sync.dma_start(
                    out=gk,
                    in_=bass.AP(tensor=bandB[2 * hp + hh],
                                offset=127 + 4 * uh * W,
                                ap=[[PIT - 1, 128], [W, 4], [1, S]]),
                )
                tiles.append(gk)
            return tiles

        def band_chunk_thunks(hp, side, t):
            # emit_band_t(big=False) split into per-chunk thunks so the
            # chunks can spread across a B2 u-step (avoids psB WAR stalls
            # blocking the PE FIFO)
            bsrc, table, bufs_ = ((qT_sb, drT_sb, bandA) if side == 0
                                  else (kT_sb, dT_sb, bandB))
            j0 = 896 - 128 * t
            th_, tl = t // 2, t % 2
            if tl == 0:
                band_stage[side] = [
                    bandsb.tile([128, 2, W], FP8, tag="bandsb",
                                name=f"bst{hp}_{side}_{th_}_{h}")
                    for h in range(2)]
            stg = band_stage[side]
            thunks = []

            def mk(c0, cn, hh):
                def go():
                    hb = 64 * hh
                    ps = psB.tile([128, 512], F32, tag="bp",
                                  name=f"psb{side}{hp}{hh}_{t}_{c0}")
                    nc.tensor.matmul(
                        ps[:, 0:cn],
                        lhsT=bsrc[hb:hb + 64, hp, 128 * t:128 * t + 128],
                        rhs=table[hb:hb + 64, j0 + c0:j0 + c0 + cn],
                        start=True, stop=True,
                    )
                    _evac(stg[hh][:, tl, c0:c0 + cn], ps[:, 0:cn])
                return go

            for (c0, cn) in ((0, 512), (512, 512), (1024, 128)):
                for hh in range(2):
                    thunks.append(mk(c0, cn, hh))
            if tl == 1:
                def wr():
                    for hh in range(2):
                        nc.sync.dma_start(
                            out=bass.AP(tensor=bufs_[2 * hp + hh],
                                        offset=th_ * 2 * W,
                                        ap=[[PIT, 128], [1, 2 * W]]),
                            in_=stg[hh],
                        )
                thunks.append(wr)
            return thunks

        # ---- attention u-step (one lh phase) ----
        def emit_scores(hp, u, lh, aex, gk, pt2, bthunks=()):
            du = u % 4
            sp = psS.tile([128, 1024], F32, tag="ps",
                          name=f"sp{hp}_{lh}_{u}")
            for hh in range(2):
                hb = 64 * hh
                sph = sp[:, 512 * hh:512 * hh + 512]
                nc.tensor.matmul(
                    sph,
                    lhsT=kT_sb[hb:hb + 64, hp, 128 * u:128 * u + 128],
                    rhs=qT_sb[hb:hb + 64, hp, 512 * lh:512 * lh + 512],
                    start=True, stop=False,
                )
                for i in range(4):
                    t = 4 * lh + i
                    nc.tensor.matmul(
                        sp[:, 512 * hh + 128 * i:512 * hh + 128 * i + 128],
                        lhsT=aex[hh][:, t, 128 * u:128 * u + 128],
                        rhs=id8_sb,
                        start=False, stop=False,
                    )
                nc.tensor.matmul(
                    sph,
                    lhsT=id8_sb,
                    rhs=gk[hh][:, du, 512 * lh:512 * lh + 512],
                    start=False, stop=True,
                )
                for th in bthunks[2 * hh:2 * hh + 2]:
                    th()
            nc.scalar.activation(
                out=pt2, in_=sp,
                func=AF.Exp, bias=mask_sb[:, u:u + 1], scale=0.125,
            )

        def emit_pv(hp, u, cps, pt2):
            for hh in range(2):
                h = 2 * hp + hh
                nc.tensor.matmul(
                    cps[hh],
                    lhsT=vv_sb[:, u, 65 * h:65 * h + 65],
                    rhs=pt2[:, hh, :],
                    start=(u == 0), stop=(u == 7),
                )

        def emit_half_tail(hp, lh, cus, critical=False):
            # z-normalize sequence-half lh of pair hp, exchange that half.
            # AG output block layout in ctx_recv[hp]: rows [256*lh ... ]:
            #   [from-rank0 (128); from-rank1 (128)]
            ctxp_t = ctxph[lh].tile([64, 2, 512], BF16, tag=f"ctxp{lh}",
                                    name=f"ctxp{hp}_{lh}")
            for hh in range(2):
                cu = cus[hh]
                # 1/Z via exp(-log Z) on ACT: ~20x faster than the
                # 8-cycle-per-element DVE reciprocal
                zlog = misc.tile([1, 512], F32, tag="zlog",
                                 name=f"zlog{2 * hp + hh}_{lh}")
                nc.scalar.activation(out=zlog,
                                     in_=cu[64:65, 512 * lh:512 * lh + 512],
                                     func=AF.Ln)
                zrow = misc.tile([1, 512], BF16, tag="zrow",
                                 name=f"zrow{2 * hp + hh}_{lh}")
                nc.scalar.activation(out=zrow, in_=zlog,
                                     func=AF.Exp, scale=-1.0)
                if critical:
                    zps = psB.tile([64, 512], F32, tag="bp",
                                   name=f"zps{2 * hp + hh}_{lh}")
                    nc.tensor.matmul(
                        zps,
                        lhsT=ones_row[0:1, :],
                        rhs=zrow,
                        start=True, stop=True,
                    )
                    nc.vector.tensor_tensor(
                        out=ctxp_t[:, hh, :],
                        in0=cu[0:64, 512 * lh:512 * lh + 512],
                        in1=zps, op=ALU.mult,
                    )
                else:
                    nc.sync.dma_start(out=zdram[hh][0:1, 0:512], in_=zrow)
                    zrec = misc.tile([64, 512], BF16, tag="zrec",
                                     name=f"zrec{2 * hp + hh}_{lh}")
                    nc.sync.dma_start(
                        out=zrec,
                        in_=bass.AP(tensor=zdram[hh], offset=0,
                                    ap=[[0, 64], [1, 512]]),
                    )
                    nc.gpsimd.tensor_tensor(
                        out=ctxp_t[:, hh, :],
                        in0=cu[0:64, 512 * lh:512 * lh + 512],
                        in1=zrec, op=ALU.mult,
                    )
                nc.sync.dma_start(
                    out=ctx_send[hp][lh][64 * hh:64 * hh + 64, :],
                    in_=ctxp_t[:, hh, :])
            nc.gpsimd.collective_compute(
                "AllGather",
                ALU.bypass,
                replica_groups=RG,
                ins=[ctx_send[hp][lh][:, :]],
                outs=[ctx_recv[hp][256 * lh:256 * lh + 256, :]],
            )

        def emit_ctx2_read(hp):
            # this rank keeps half g: its blocks sit at rows 256*g + {0,128}
            nc.sync.dma_start(
                out=ctx2[:, hp, :],
                in_=ctx_recv[hp][bass.ds(goff2, 128), :])
            nc.sync.dma_start(
                out=ctx2[:, 4 + hp, :],
                in_=ctx_recv[hp][bass.ds(goff2 + 128, 128), :])

        # rank-within-pair row offset for the exchange read-back
        goff2 = (nc.sync.partition_id() % 2) * 256

        # ================= emission schedule =================
        # prologue: dense proj with bands(0,1) riding on their own PSUM
        proj_thunks = [
            (proj_block, (wq_sb, bq_sb, qT_sb, 0)),
            (proj_block, (wq_sb, bq_sb, qT_sb, 1)),
            (proj_block, (wq_sb, bq_sb, qT_sb, 2)),
            (proj_block, (wq_sb, bq_sb, qT_sb, 3)),
            (proj_block, (wk_sb, bk_sb, kT_sb, 0)),
            (proj_block, (wk_sb, bk_sb, kT_sb, 1)),
            (proj_block, (wk_sb, bk_sb, kT_sb, 2)),
            (proj_block, (wk_sb, bk_sb, kT_sb, 3)),
        ]

        def run(th):
            th[0](*th[1])

        run(proj_thunks[0]); run(proj_thunks[1])
        filler = [proj_thunks[2], proj_thunks[3], proj_thunks[4],
                  proj_thunks[5]]
        fi = 0
        for t in range(BT):
            emit_band_t(0, 0, t, big=True)
            if t % 2 == 1 and fi < len(filler):
                run(filler[fi]); fi += 1
        filler = [proj_thunks[6], proj_thunks[7]]
        fi = 0
        for t in range(BT):
            emit_band_t(0, 1, t, big=True)
            if t % 4 == 3 and fi < len(filler):
                run(filler[fi]); fi += 1
        wv_sb = wpool.tile([128, 8, 520], BF16, tag="w", name="wv_sb")
        for kc in range(8):
            nc.sync.dma_start(out=wv_sb[:, kc, :],
                              in_=wvT[128 * kc:128 * kc + 128, :])
        aex_cur = issue_aex(0)
        gk_cur = [issue_gk(0, 0), issue_gk(0, 1)]
        vi = 0
        for side in range(2):
            for t in range(BT):
                emit_band_t(1, side, t, big=True)
                if t % 2 == 1 and vi < 8:
                    emit_v_block(vi); vi += 1
        while vi < 8:
            emit_v_block(vi); vi += 1
        aex_nxt = issue_aex(1)
        gk_nxt = [issue_gk(1, 0), issue_gk(1, 1)]

        # main pipeline: B2(p) (lh phases) with bands(p+2) on psB banks
        evac_rr[:] = [1, 0, 1, 1, 1, 0, 1, 1, 0]  # ACT 3/9 in B2 (owns exps)
        wo_sb = [None, None]
        res_sb = [None] * 4
        pending_tail = None
        pending_read = None
        pending_read2 = []

        for p in range(4):
            band_p = p + 2
            cus = []
            for hh in range(2):
                cus.append(ctxup.tile([128, S], BF16, tag="ctxu",
                                      name=f"ctxU{2 * p + hh}"))
            for lh in range(2):
                cps = [psC.tile([65, 512], F32, tag=f"c{hh}",
                                name=f"cps{p}_{lh}_{hh}") for hh in range(2)]
                pt_prev = None
                for u in range(BT):
                    if u == 1 and pending_tail is not None:
                        emit_half_tail(*pending_tail)
                        pending_tail = None
                    if u == 3 and pending_read is not None:
                        emit_ctx2_read(pending_read)
                        pending_read = None
                    if lh == 0 and u == 5 and pending_read2:
                        emit_ctx2_read(pending_read2.pop(0))
                    pt2 = ppool.tile([128, 2, 512], BF16, tag="pt",
                                     name=f"pt{p}_{lh}_{u}")
                    bthunks = (band_chunk_thunks(band_p, lh, u)
                               if band_p <= 3 else [])
                    emit_scores(p, u, lh, aex_cur, gk_cur[u // 4], pt2,
                                bthunks)
                    if pt_prev is not None:
                        emit_pv(p, u - 1, cps, pt_prev)
                    for th in bthunks[4:]:
                        th()
                    pt_prev = pt2
                    if band_p <= 3:
                        pass
                    elif p == 2 and lh == 0:
                        if u % 4 == 0:
                            half = u // 4
                            wo_sb[half] = wpool.tile(
                                [128, 4, 1024], BF16, tag="w",
                                name=f"wo_sb{half}")
                        nc.sync.dma_start(
                            out=wo_sb[u // 4][:, u % 4, :],
                            in_=woT[128 * u:128 * u + 128, :])
                        if u % 2 == 0:
                            lt = u // 2
                            rsb = bigp.tile([128, HID], BF16, tag="rsb",
                                            name=f"rsb{lt}", bufs=4)
                            nc.sync.dma_start(
                                out=rsb, in_=res[128 * lt:128 * lt + 128, :])
                            res_sb[lt] = rsb
                emit_pv(p, 7, cps, pt_prev)
                # evacuate PV accumulators for this lh phase (frees psC)
                for hh in range(2):
                    nc.vector.tensor_copy(
                        out=cus[hh][0:65, 512 * lh:512 * lh + 512],
                        in_=cps[hh])
                if p == 3:
                    emit_half_tail(p, lh, cus, critical=(lh == 1))
                elif lh == 1:
                    pending_tail = (p, 1, cus)
                else:
                    emit_half_tail(p, 0, cus)
            if p == 3:
                emit_ctx2_read(3)
            else:
                pending_read = p if p == 2 else None
                if p < 2:
                    pending_read2.append(p)
            aex_cur, gk_cur = aex_nxt, gk_nxt
            if band_p <= 3:
                aex_nxt = issue_aex(band_p)
                gk_nxt = [issue_gk(band_p, 0), issue_gk(band_p, 1)]

        # ---- output dense (full K) + residual + LayerNorm on seq half ----
        KC_EARLY = [0, 1, 2, 4, 5, 6]
        KC_LATE = [3, 7]
        wo_open = {}

        def wo_early(lt):
            ps = psS.tile([128, 1024], F32, tag="ps", name=f"ps_o_{lt}")
            for nh2 in range(2):
                for kc in KC_EARLY:
                    nc.tensor.matmul(
                        ps[:, 512 * nh2:512 * nh2 + 512],
                        lhsT=ctx2[:, kc, 128 * lt:128 * lt + 128],
                        rhs=wo_sb[kc // 4][:, kc % 4, 512 * nh2:512 * nh2 + 512],
                        start=(kc == 0), stop=False,
                    )
            wo_open[lt] = ps

        wo_early(0)
        wo_early(1)
        for lt in range(4):
            ps = wo_open.pop(lt)
            for nh2 in range(2):
                for kc in KC_LATE:
                    nc.tensor.matmul(
                        ps[:, 512 * nh2:512 * nh2 + 512],
                        lhsT=ctx2[:, kc, 128 * lt:128 * lt + 128],
                        rhs=wo_sb[kc // 4][:, kc % 4, 512 * nh2:512 * nh2 + 512],
                        start=False, stop=(kc == 7),
                    )
            h2 = bigp.tile([128, HID], BF16, tag="h2", name=f"h2_{lt}")
            nc.vector.tensor_tensor(out=h2, in0=ps, in1=res_sb[lt],
                                    op=ALU.add)
            if lt + 2 < 4:
                wo_early(lt + 2)
            stat = lns.tile([128, 16], F32, tag="stat", name=f"stat{lt}")
            for c in range(2):
                nc.vector.bn_stats(out=stat[:, 6 * c:6 * c + 6],
                                   in_=h2[:, 512 * c:512 * c + 512])
            mv = lns.tile([128, 4], F32, tag="mv", name=f"mv{lt}")
            nc.vector.bn_aggr(out=mv[:, 0:2],
                              in_=stat[:, 0:12].rearrange("p (n s) -> p n s", n=2))
            nc.scalar.activation(out=mv[:, 2:3], in_=mv[:, 1:2],
                                 func=AF.Sqrt, bias=eps_sb, scale=1.0)
            nc.vector.reciprocal(out=mv[:, 3:4], in_=mv[:, 2:3])

            xn = bigp.tile([128, HID], BF16, tag="xn", name=f"xn{lt}")
            nc.vector.tensor_scalar(
                out=xn, in0=h2,
                scalar1=mv[:, 0:1], scalar2=mv[:, 3:4],
                op0=ALU.subtract, op1=ALU.mult,
            )
            xg = bigp.tile([128, HID], BF16, tag="xn", name=f"xg{lt}")
            nc.vector.tensor_tensor(out=xg, in0=xn, in1=lng_bc, op=ALU.mult)
            ob = bigp.tile([128, HID], F32, tag="ob", name=f"ob{lt}")
            nc.vector.tensor_tensor(out=ob, in0=xg, in1=lnb_bc, op=ALU.add)
            nc.sync.dma_start(out=out[128 * lt:128 * lt + 128, :], in_=ob)

    nc.compile()
    return nc


def make_in_maps(hidden_states, attention_mask, Wq, bq, Wk, bk, Wv, bv,
                 dist_emb, Wo, bo, ln_g, ln_b):
    bf16 = ml_dtypes.bfloat16
    hs = np.ascontiguousarray(hidden_states, dtype=np.float32)
    mask = np.ascontiguousarray(attention_mask, dtype=np.float32)
    Wq = np.asarray(Wq, np.float32); Wk = np.asarray(Wk, np.float32)
    Wv = np.asarray(Wv, np.float32); Wo = np.asarray(Wo, np.float32)
    bq = np.asarray(bq, np.float32); bk = np.asarray(bk, np.float32)
    bv = np.asarray(bv, np.float32); bo = np.asarray(bo, np.float32)
    D = np.asarray(dist_emb, np.float32)
    ln_g = np.asarray(ln_g, np.float32); ln_b = np.asarray(ln_b, np.float32)

    z1 = np.zeros((1, HD), np.float32)
    dT = np.tile(np.concatenate([D, z1], 0).T, (2, 1)).astype(bf16)
    drT = np.tile(np.concatenate([D[::-1], z1], 0).T, (2, 1)).astype(bf16)
    ident8 = np.eye(128, dtype=np.float32).astype(ml_dtypes.float8_e4m3)
    woT = np.ascontiguousarray(Wo.T.astype(bf16))

    in_maps = []
    for core in range(NCORES):
        b, g = core // 2, core % 2
        sl = slice(512 * g, 512 * g + 512)
        wvT_aug = np.zeros((HID, 520), np.float32)
        bv_aug = np.zeros(520, np.float32)
        for h in range(8):
            cs = 512 * g + 64 * h
            wvT_aug[:, 65 * h:65 * h + 64] = Wv[cs:cs + 64].T
            bv_aug[65 * h:65 * h + 64] = bv[cs:cs + 64]
            bv_aug[65 * h + 64] = 1.0
        in_maps.append({
            "hsT": np.ascontiguousarray(hs[b].T).astype(bf16),
            "res": np.ascontiguousarray(
                hs[b, 512 * g:512 * g + 512] + bo[None, :]).astype(bf16),
            "wqT": np.ascontiguousarray(Wq[sl].T).astype(bf16),
            "wkT": np.ascontiguousarray(Wk[sl].T).astype(bf16),
            "wvT": wvT_aug.astype(bf16),
            "bq": np.ascontiguousarray(bq[sl].reshape(4, 128).T),
            "bk": np.ascontiguousarray(bk[sl].reshape(4, 128).T),
            "bvaug": bv_aug,
            "drT": drT,
            "dT": dT,
            "woT": woT,
            "maskc": np.ascontiguousarray(mask[b, 0, 0].reshape(8, 128).T),
            "ident8": ident8,
            "ones64": np.ones((128, 64), np.float32).astype(bf16),
            "lng": ln_g.astype(bf16),
            "lnb": ln_b.astype(bf16),
        })
    return in_maps


def kernel(**inputs):
    global _COMPILED
    if _COMPILED is None:
        _COMPILED = build_program()
    nc = _COMPILED
    in_maps = make_in_maps(**inputs)
    result = run_bass_kernel_spmd(nc, in_maps, core_ids=list(range(NCORES)))
    out = np.zeros((B, S, HID), np.float32)
    for core in range(NCORES):
        b, g = core // 2, core % 2
        out[b, 512 * g:512 * g + 512] = result.results[core]["out"]
    return out
